# Initial kernel scaffold
#
"""Carrier-frequency-offset rotation kernel for 8 Trainium2 NeuronCores.

out[0] = x_real*cos(ang) - x_imag*sin(ang)
out[1] = x_real*sin(ang) + x_imag*cos(ang)
ang[n] = 2*pi*n*w_delta/Fs, Fs = 64e9, per column n (shared by all batch rows).

Sharding: pure data parallel over the batch dim — core k handles rows
[8k, 8k+8) of the [64, 262144] inputs. The length-N phase vectors are
computed on-device per core (iota -> u = n*rate -> frac = u - rint(u) ->
sin(2*pi*frac)), which is cheaper than DMAing them in.
"""

import numpy as np

import concourse.bacc as bacc
import concourse.mybir as mybir
from concourse.tile import TileContext
from concourse.bass_utils import run_bass_kernel_spmd

FS = 64e9
B, N = 64, 262144
P, F = 128, 2048          # one row = [128 partitions, 2048 free] = 1 MiB fp32
NCORES = 8
RB = B // NCORES          # rows per core

f32 = mybir.dt.float32
i32 = mybir.dt.int32
Sin = mybir.ActivationFunctionType.Sin
Alu = mybir.AluOpType
TWO_PI = float(np.float32(2.0 * np.pi))

LAST_RESULT = None        # BassKernelResults of the most recent run (for test.py)
_BUILD_CACHE = {}


def _build(rate: float):
    """Build the single-core SPMD program. `rate` = w_delta/Fs (fp32 value)."""
    nc = bacc.Bacc()
    xr_h = nc.declare_dram_parameter("xr", [RB, P, F], f32, isOutput=False)
    xi_h = nc.declare_dram_parameter("xi", [RB, P, F], f32, isOutput=False)
    ore_h = nc.declare_dram_parameter("o_re", [RB, P, F], f32, isOutput=True)
    oim_h = nc.declare_dram_parameter("o_im", [RB, P, F], f32, isOutput=True)

    with TileContext(nc) as tc:
        with tc.tile_pool(name="phase", bufs=1) as phase_pool:
            c_t = phase_pool.tile([P, F], f32, name="c_t")
            s_t = phase_pool.tile([P, F], f32, name="s_t")

            with tc.tile_pool(name="setup", bufs=1) as sp:
                n_i = sp.tile([P, F], i32, name="n_i")
                nc.gpsimd.iota(n_i, pattern=[[1, F]], base=0, channel_multiplier=F)
                n_f = sp.tile([P, F], f32, name="n_f")
                nc.vector.tensor_copy(out=n_f, in_=n_i)
                for phase_t, shift in ((s_t, 0.0), (c_t, 0.25)):
                    u = sp.tile([P, F], f32, name="u", tag="u")
                    if shift:
                        nc.vector.tensor_scalar(u, n_f, rate, shift, Alu.mult, Alu.add)
                    else:
                        nc.vector.tensor_scalar_mul(u, n_f, rate)
                    k_i = sp.tile([P, F], i32, name="k_i", tag="k_i")
                    nc.vector.tensor_copy(out=k_i, in_=u)     # rint (round-to-nearest)
                    k_f = sp.tile([P, F], f32, name="k_f", tag="k_f")
                    nc.vector.tensor_copy(out=k_f, in_=k_i)
                    frac = sp.tile([P, F], f32, name="frac", tag="frac")
                    nc.vector.tensor_sub(out=frac, in0=u, in1=k_f)
                    nc.scalar.activation(phase_t, frac, Sin, scale=TWO_PI)

            with tc.tile_pool(name="io", bufs=2) as pool:
                for r in range(RB):
                    xr_t = pool.tile([P, F], f32, tag="xr", name="xr_t")
                    xi_t = pool.tile([P, F], f32, tag="xi", name="xi_t")
                    nc.sync.dma_start(out=xr_t, in_=xr_h[r])
                    nc.sync.dma_start(out=xi_t, in_=xi_h[r])
                    m1 = pool.tile([P, F], f32, tag="m1", name="m1")
                    m2 = pool.tile([P, F], f32, tag="m2", name="m2")
                    m3 = pool.tile([P, F], f32, tag="m3", name="m3")
                    m4 = pool.tile([P, F], f32, tag="m4", name="m4")
                    nc.vector.tensor_mul(out=m1, in0=xr_t, in1=c_t)
                    nc.any.tensor_mul(out=m2, in0=xi_t, in1=s_t)
                    nc.any.tensor_mul(out=m3, in0=xr_t, in1=s_t)
                    nc.gpsimd.tensor_mul(out=m4, in0=xi_t, in1=c_t)
                    nc.vector.tensor_sub(out=m1, in0=m1, in1=m2)   # out_real
                    nc.any.tensor_add(out=m3, in0=m3, in1=m4)      # out_imag
                    nc.sync.dma_start(out=ore_h[r], in_=m1)
                    nc.sync.dma_start(out=oim_h[r], in_=m3)
    nc.compile()
    return nc


def kernel(x_real, x_imag, w_delta):
    global LAST_RESULT
    x_real = np.ascontiguousarray(np.asarray(x_real), dtype=np.float32)
    x_imag = np.ascontiguousarray(np.asarray(x_imag), dtype=np.float32)
    w_delta = np.asarray(w_delta, dtype=np.float32)

    rate = float(np.float32(w_delta[0]) / np.float32(FS))
    if rate not in _BUILD_CACHE:
        _BUILD_CACHE[rate] = _build(rate)
    nc = _BUILD_CACHE[rate]

    in_maps = []
    for k in range(NCORES):
        rows = slice(k * RB, (k + 1) * RB)
        in_maps.append({
            "xr": np.ascontiguousarray(x_real[rows]).reshape(RB, P, F),
            "xi": np.ascontiguousarray(x_imag[rows]).reshape(RB, P, F),
        })

    LAST_RESULT = run_bass_kernel_spmd(nc, in_maps, core_ids=list(range(NCORES)))

    out = np.empty((2, B, N), dtype=np.float32)
    for k, res in enumerate(LAST_RESULT.results):
        rows = slice(k * RB, (k + 1) * RB)
        out[0, rows] = res["o_re"].reshape(RB, N)
        out[1, rows] = res["o_im"].reshape(RB, N)
    return out


# revision 3
# speedup vs baseline: 6.7516x; 6.7516x over previous
"""Carrier-frequency-offset rotation kernel for 8 Trainium2 NeuronCores.

out[0] = x_real*cos(ang) - x_imag*sin(ang)
out[1] = x_real*sin(ang) + x_imag*cos(ang)
ang[n] = 2*pi*n*w_delta/Fs, Fs = 64e9, per column n (shared by all batch rows).

Sharding: pure data parallel over the batch dim — core k handles rows
[8k, 8k+8) of the [64, 262144] inputs. The length-N phase vectors are
computed on-device per core (iota -> u = n*rate -> frac = u - rint(u) ->
sin(2*pi*frac)), which is cheaper than DMAing them in.
"""

import numpy as np

import concourse.bacc as bacc
import concourse.mybir as mybir
from concourse.tile import TileContext
from concourse.bass_utils import run_bass_kernel_spmd

FS = 64e9
B, N = 64, 262144
P, F = 128, 2048          # one row = [128 partitions, 2048 free] = 1 MiB fp32
NCORES = 8
RB = B // NCORES          # rows per core

f32 = mybir.dt.float32
i32 = mybir.dt.int32
Sin = mybir.ActivationFunctionType.Sin
Alu = mybir.AluOpType
TWO_PI = float(np.float32(2.0 * np.pi))

LAST_RESULT = None        # BassKernelResults of the most recent run (for test.py)
_BUILD_CACHE = {}


def _build(rate: float, repeats: int = 1):
    """Build the single-core SPMD program. `rate` = w_delta/Fs (fp32 value).

    `repeats` re-runs the row pipeline that many times (same data, same
    result) — used only for differential HW timing from test/bench scripts.
    """
    nc = bacc.Bacc()
    xr_h = nc.declare_dram_parameter("xr", [RB, P, F], f32, isOutput=False)
    xi_h = nc.declare_dram_parameter("xi", [RB, P, F], f32, isOutput=False)
    ore_h = nc.declare_dram_parameter("o_re", [RB, P, F], f32, isOutput=True)
    oim_h = nc.declare_dram_parameter("o_im", [RB, P, F], f32, isOutput=True)

    with TileContext(nc) as tc:
        with tc.tile_pool(name="phase", bufs=1) as phase_pool:
            c_t = phase_pool.tile([P, F], f32, name="c_t")
            s_t = phase_pool.tile([P, F], f32, name="s_t")

            with tc.tile_pool(name="setup", bufs=1) as sp:
                n_i = sp.tile([P, F], i32, name="n_i")
                nc.gpsimd.iota(n_i, pattern=[[1, F]], base=0, channel_multiplier=F)
                n_f = sp.tile([P, F], f32, name="n_f")
                nc.vector.tensor_copy(out=n_f, in_=n_i)
                for phase_t, shift in ((s_t, 0.0), (c_t, 0.25)):
                    u = sp.tile([P, F], f32, name="u", tag="u")
                    if shift:
                        nc.vector.tensor_scalar(u, n_f, rate, shift, Alu.mult, Alu.add)
                    else:
                        nc.vector.tensor_scalar_mul(u, n_f, rate)
                    k_i = sp.tile([P, F], i32, name="k_i", tag="k_i")
                    nc.vector.tensor_copy(out=k_i, in_=u)     # rint (round-to-nearest)
                    k_f = sp.tile([P, F], f32, name="k_f", tag="k_f")
                    nc.vector.tensor_copy(out=k_f, in_=k_i)
                    frac = sp.tile([P, F], f32, name="frac", tag="frac")
                    nc.vector.tensor_sub(out=frac, in0=u, in1=k_f)
                    nc.scalar.activation(phase_t, frac, Sin, scale=TWO_PI)

            with tc.tile_pool(name="io", bufs=2) as pool:
                for r in [r for _ in range(repeats) for r in range(RB)]:
                    xr_t = pool.tile([P, F], f32, tag="xr", name="xr_t")
                    xi_t = pool.tile([P, F], f32, tag="xi", name="xi_t")
                    nc.sync.dma_start(out=xr_t, in_=xr_h[r])
                    nc.sync.dma_start(out=xi_t, in_=xi_h[r])
                    m1 = pool.tile([P, F], f32, tag="m1", name="m1")
                    m2 = pool.tile([P, F], f32, tag="m2", name="m2")
                    m3 = pool.tile([P, F], f32, tag="m3", name="m3")
                    m4 = pool.tile([P, F], f32, tag="m4", name="m4")
                    nc.vector.tensor_mul(out=m1, in0=xr_t, in1=c_t)
                    nc.any.tensor_mul(out=m2, in0=xi_t, in1=s_t)
                    nc.any.tensor_mul(out=m3, in0=xr_t, in1=s_t)
                    nc.gpsimd.tensor_mul(out=m4, in0=xi_t, in1=c_t)
                    nc.vector.tensor_sub(out=m1, in0=m1, in1=m2)   # out_real
                    nc.any.tensor_add(out=m3, in0=m3, in1=m4)      # out_imag
                    nc.sync.dma_start(out=ore_h[r], in_=m1)
                    nc.sync.dma_start(out=oim_h[r], in_=m3)
    nc.compile()
    return nc


def kernel(x_real, x_imag, w_delta):
    global LAST_RESULT
    x_real = np.ascontiguousarray(np.asarray(x_real), dtype=np.float32)
    x_imag = np.ascontiguousarray(np.asarray(x_imag), dtype=np.float32)
    w_delta = np.asarray(w_delta, dtype=np.float32)

    rate = float(np.float32(w_delta[0]) / np.float32(FS))
    if rate not in _BUILD_CACHE:
        _BUILD_CACHE[rate] = _build(rate)
    nc = _BUILD_CACHE[rate]

    in_maps = []
    for k in range(NCORES):
        rows = slice(k * RB, (k + 1) * RB)
        in_maps.append({
            "xr": np.ascontiguousarray(x_real[rows]).reshape(RB, P, F),
            "xi": np.ascontiguousarray(x_imag[rows]).reshape(RB, P, F),
        })

    LAST_RESULT = run_bass_kernel_spmd(nc, in_maps, core_ids=list(range(NCORES)))

    out = np.empty((2, B, N), dtype=np.float32)
    for k, res in enumerate(LAST_RESULT.results):
        rows = slice(k * RB, (k + 1) * RB)
        out[0, rows] = res["o_re"].reshape(RB, N)
        out[1, rows] = res["o_im"].reshape(RB, N)
    return out
